# revision 3
# baseline (speedup 1.0000x reference)
"""JPEG layer (nn_JpegLayer) Trainium2 Bass kernel, 8-core data parallel.

Pipeline per image (per core: 4 images of [3,512,512]):
  P1: 3-accum matmuls fold RGB->YCC color mix + H-DCT (+ vertical 2x-pool for
      chroma) ; route-A, out [h'freq, w]
  T1: PE transposes -> [w, h'freq]
  P2: W-DCT (+ horizontal pool fold for chroma) + DC level-shift correction
      via an extra accumulated rank-structured matmul -> coeffs [w'', h']
  Q : e = d*(1/q); round via +/-2^23*1.5 trick; dec = r*q   (DVE/GPSIMD)
  P3: W-IDCT (+ horizontal 2x upsample fold for chroma) -> [w, h']
  T2: PE transposes -> [h', w]
  P4: H-IDCT (+ vertical upsample fold for chroma) + YCC->RGB fold via
      accumulated matmuls + LEVEL plane via ones-matmul -> psum RGB
  out: DVE tensor_scalar (max 0, min 1) psum->sbuf, DMA out.

All matmul data is float32r (TRN2 reduced-precision fp32 path, 1 cyc/row at
N>=256). Forward-path rounding error ~1e-4 rel; set FP32_FWD=True to run the
forward passes in full fp32 (4 cyc/row) if more accuracy is needed.
"""
import sys
sys.path.insert(0, '/opt/trn_rl_repo')
import numpy as np
import concourse.bacc as bacc
import concourse.bass as bass
import concourse.mybir as mybir
import concourse.tile as tile
from concourse import bass_utils

N_CORES = 8
IMG_PER_CORE = 4
H = W = 512
HT = H // 128            # 4 h-tiles per plane
LEVEL = np.float32(128.0 / 255.0)
LEVEL_F = float(LEVEL)
C_ROUND = 12582912.0   # 1.5*2^23: (x+C)-C == round-half-even(x)
F32 = mybir.dt.float32
F32R = mybir.dt.float32r

RGB2YCC = np.array([[0.299, 0.587, 0.114],
                    [-0.168735892, -0.331264108, 0.5],
                    [0.5, -0.418687589, -0.081312411]], dtype=np.float32)
# YCC2RGB columns: Y col = [1,1,1]; cb col = [0,-0.344136286,1.772]; cr col = [1.402,-0.714136286,0]
CB_C = np.array([0.0, -0.344136286, 1.772], dtype=np.float32)
CR_C = np.array([1.402, -0.714136286, 0.0], dtype=np.float32)


def _dct8():
    i = np.arange(8)[:, None].astype(np.float64)
    j = np.arange(8)[None, :].astype(np.float64)
    m = np.sqrt(2.0 / 8) * np.cos(np.pi * (2 * j + 1) * i / 16.0)
    m[0, :] = 1.0 / np.sqrt(8.0)
    return m.astype(np.float32)


def _blockdiag(b, reps):
    r, c = b.shape
    out = np.zeros((r * reps, c * reps), dtype=np.float32)
    for k in range(reps):
        out[k * r:(k + 1) * r, k * c:(k + 1) * c] = b
    return out


def _build_consts(quantize):
    D = _dct8()
    BD_T = _blockdiag(D.T, 16)             # [128,128] fwd 1D-DCT as lhsT
    BD = _blockdiag(D, 16)                 # [128,128] inverse
    # pooled fwd: PF[16b+2ii+dh, 8b+u] = D[u,ii]/2    [128, 64]
    pf8 = np.zeros((16, 8), dtype=np.float32)
    for ii in range(8):
        for dh in range(2):
            pf8[2 * ii + dh, :] = D[:, ii] * 0.5
    PF = _blockdiag(pf8, 8)                # [128, 64]
    # upsample inverse: PU[8b+v, 16b+2jj+dw] = D[v,jj]   [64, 128]
    pu8 = np.zeros((8, 16), dtype=np.float32)
    for jj in range(8):
        for dw in range(2):
            pu8[:, 2 * jj + dw] = D[jj, :]     # D.T[v,jj] = D[jj,v]? no:
    # careful: idct y[j] = sum_v D[v,j] z[v]  => PU[v, col(j,dw)] = D[v, j]
    pu8 = np.zeros((8, 16), dtype=np.float32)
    for jj in range(8):
        for dw in range(2):
            pu8[:, 2 * jj + dw] = D[:, jj]
    PU = _blockdiag(pu8, 8)                # [64, 128]

    consts = {}
    for c in range(3):
        consts[f"w1y{c}"] = RGB2YCC[0, c] * BD_T
        consts[f"w1c{c}"] = np.concatenate(
            [RGB2YCC[1, c] * PF, RGB2YCC[2, c] * PF], axis=1)  # [128,128]
    consts["w2y"] = BD_T
    consts["w2c"] = PF                     # [128, 64]
    consts["w3y"] = BD
    consts["w3c"] = PU                     # [64, 128]
    consts["w4y"] = BD
    w4 = {}
    for name, cb, cr in (("R", CB_C[0], CR_C[0]), ("G", CB_C[1], CR_C[1]),
                         ("B", CB_C[2], CR_C[2])):
        m = np.zeros((128, 128), dtype=np.float32)
        m[0:64, :] = cb * PU
        m[64:128, :] = cr * PU
        consts[f"w4c{name}"] = m
    consts["ident"] = np.eye(128, dtype=np.float32)

    # quant tables: q = round(quantize[0]*255)/255 (f32, all channels)
    q = (np.round(quantize[0].astype(np.float32) * np.float32(255.0))
         / np.float32(255.0)).astype(np.float32)
    rq = (1.0 / q.astype(np.float64)).astype(np.float32)
    consts["rqt"] = np.tile(rq.T, (16, 64)).astype(np.float32)   # [128,512]
    consts["qt"] = np.tile(q.T, (16, 64)).astype(np.float32)
    # DC correction: coeff d_true = d - 8L*delta00. Via accumulated matmul:
    # lhsT dccor [128,128]: col p (p%8==0) = -8L/128 ; rhs pat8 [128,512]:
    # pat8[k, n] = 1 if n%8==0 else 0  -> psum[p,n] += -8L*d(p%8=0)*d(n%8=0)
    dccor = np.zeros((128, 128), dtype=np.float32)
    dccor[:, 0::8] = np.float32(-8.0 * LEVEL / 128.0)
    consts["dccor"] = dccor
    pat8 = np.zeros((128, 512), dtype=np.float32)
    pat8[:, 0::8] = 1.0
    consts["pat8"] = pat8
    # LEVEL plane: lhsT lones [128,128] all L/128, rhs ones [128,512]
    consts["lones"] = np.full((128, 128), LEVEL / np.float32(128.0),
                              dtype=np.float32)
    consts["ones"] = np.ones((128, 512), dtype=np.float32)
    return consts


_CONST_SHAPES = None


def _build_nc():
    nc = bacc.Bacc("TRN2", target_bir_lowering=False, debug=False,
                   enable_asserts=False, num_devices=N_CORES)
    x_d = nc.dram_tensor("x", [IMG_PER_CORE, 3, H, W], F32R,
                         kind="ExternalInput").ap()
    out_d = nc.dram_tensor("out", [IMG_PER_CORE, 3, H, W], F32,
                           kind="ExternalOutput").ap()
    cd = {}
    for name, shape in _CONST_SHAPES.items():
        cd[name] = nc.dram_tensor(name, list(shape), F32R,
                                  kind="ExternalInput").ap()

    with tile.TileContext(nc) as tc:
        with tc.tile_pool(name="consts", bufs=1) as cp, \
             tc.tile_pool(name="xin", bufs=14) as xp, \
             tc.tile_pool(name="work", bufs=5) as wp, \
             tc.tile_pool(name="stage", bufs=4) as sp, \
             tc.tile_pool(name="psmm", bufs=2, space="PSUM") as pmm, \
             tc.tile_pool(name="pstp", bufs=2, space="PSUM") as ptp:

            cs = {}
            for name, shape in _CONST_SHAPES.items():
                cs[name] = cp.tile(list(shape), F32R, tag=f"c_{name}", name=f"c_{name}")
                nc.sync.dma_start(cs[name][:], cd[name])

            ACT = mybir.ActivationFunctionType
            OP = mybir.AluOpType

            for img in range(IMG_PER_CORE):
                # ---- load RGB tiles ----
                X = {}
                for c in range(3):
                    for t in range(HT):
                        xt = xp.tile([128, 512], F32R, tag="x", name=f"x_{img}_{c}_{t}")
                        nc.sync.dma_start(
                            xt[:], x_d[img, c, 128 * t:128 * (t + 1), :])
                        X[c, t] = xt

                # ---- P1: color + H-DCT (+v-pool chroma) ----
                d1y, d1c = [], []
                for t in range(HT):
                    psY = pmm.tile([128, 512], F32, tag="mm", name="psmm_t")
                    for c in range(3):
                        nc.tensor.matmul(psY[:], cs[f"w1y{c}"][:], X[c, t][:],
                                         start=(c == 0), stop=(c == 2))
                    ty = wp.tile([128, 512], F32R, tag="d1y", name=f"d1y_{img}_{t}")
                    nc.scalar.activation(ty[:], psY[:], ACT.Copy)
                    d1y.append(ty)
                    psC = pmm.tile([128, 512], F32, tag="mm", name="psmm_t")
                    for c in range(3):
                        nc.tensor.matmul(psC[:], cs[f"w1c{c}"][:], X[c, t][:],
                                         start=(c == 0), stop=(c == 2))
                    tcc = wp.tile([128, 512], F32R, tag="d1c", name=f"d1c_{img}_{t}")
                    nc.vector.tensor_copy(tcc[:], psC[:])
                    d1c.append(tcc)

                # ---- T1 ----
                t1y, t1c = [], []
                for s in range(4):
                    pty = ptp.tile([128, 512], F32R, tag="tp", name="pstp_t")
                    for t in range(HT):
                        nc.tensor.transpose(
                            pty[:, 128 * t:128 * (t + 1)],
                            d1y[t][:, 128 * s:128 * (s + 1)], cs["ident"][:])
                    sy = wp.tile([128, 512], F32R, tag="t1y", name=f"t1y_{img}_{s}")
                    nc.scalar.activation(sy[:], pty[:], ACT.Copy)
                    t1y.append(sy)
                    ptc = ptp.tile([128, 512], F32R, tag="tp", name="pstp_t")
                    for t in range(HT):
                        nc.tensor.transpose(
                            ptc[:, 128 * t:128 * (t + 1)],
                            d1c[t][:, 128 * s:128 * (s + 1)], cs["ident"][:])
                    sc = wp.tile([128, 512], F32R, tag="t1c", name=f"t1c_{img}_{s}")
                    nc.vector.tensor_copy(sc[:], ptc[:])
                    t1c.append(sc)

                # ---- P2 + quantize ----
                decy, decc = [], []
                for s in range(4):
                    ps = pmm.tile([128, 512], F32, tag="mm", name="psmm_t")
                    nc.tensor.matmul(ps[:], cs["w2y"][:], t1y[s][:],
                                     start=True, stop=False)
                    nc.tensor.matmul(ps[:], cs["dccor"][:], cs["pat8"][:],
                                     start=False, stop=True)
                    ey = wp.tile([128, 512], F32R, tag="ey", name=f"ey_{img}_{s}")
                    nc.vector.tensor_tensor(ey[:], ps[:], cs["rqt"][:], OP.mult)
                    nc.gpsimd.tensor_scalar(ey[:], ey[:], C_ROUND, C_ROUND,
                                            OP.add, OP.subtract)
                    dy = wp.tile([128, 512], F32R, tag="decy", name=f"decy_{img}_{s}")
                    nc.vector.tensor_tensor(dy[:], ey[:], cs["qt"][:], OP.mult)
                    decy.append(dy)

                    psc = pmm.tile([64, 512], F32, tag="mmc", name="psmmc_t")
                    nc.tensor.matmul(psc[:], cs["w2c"][:], t1c[s][:],
                                     start=True, stop=True)
                    ec = wp.tile([64, 512], F32R, tag="ec", name=f"ec_{img}_{s}")
                    nc.vector.tensor_tensor(ec[:], psc[:], cs["rqt"][0:64, :],
                                            OP.mult)
                    nc.gpsimd.tensor_scalar(ec[:], ec[:], C_ROUND, C_ROUND,
                                            OP.add, OP.subtract)
                    dc = wp.tile([64, 512], F32R, tag="decc", name=f"decc_{img}_{s}")
                    nc.vector.tensor_tensor(dc[:], ec[:], cs["qt"][0:64, :],
                                            OP.mult)
                    decc.append(dc)

                # ---- P3 ----
                p3y, p3c = [], []
                for s in range(4):
                    ps = pmm.tile([128, 512], F32, tag="mm", name="psmm_t")
                    nc.tensor.matmul(ps[:], cs["w3y"][:], decy[s][:],
                                     start=True, stop=True)
                    vy = wp.tile([128, 512], F32R, tag="p3y", name=f"p3y_{img}_{s}")
                    nc.scalar.activation(vy[:], ps[:], ACT.Copy)
                    p3y.append(vy)
                    psc = pmm.tile([128, 512], F32, tag="mm", name="psmm_t")
                    nc.tensor.matmul(psc[:], cs["w3c"][:], decc[s][:],
                                     start=True, stop=True)
                    vc = wp.tile([128, 512], F32R, tag="p3c", name=f"p3c_{img}_{s}")
                    nc.scalar.activation(vc[:], psc[:], ACT.Copy)
                    p3c.append(vc)

                # ---- T2 ----
                t2y, t2c = [], []
                for t in range(4):
                    pty = ptp.tile([128, 512], F32R, tag="tp", name="pstp_t")
                    for s in range(4):
                        nc.tensor.transpose(
                            pty[:, 128 * s:128 * (s + 1)],
                            p3y[s][:, 128 * t:128 * (t + 1)], cs["ident"][:])
                    sy = wp.tile([128, 512], F32R, tag="t2y", name=f"t2y_{img}_{t}")
                    nc.scalar.activation(sy[:], pty[:], ACT.Copy)
                    t2y.append(sy)
                    ptc = ptp.tile([128, 512], F32R, tag="tp", name="pstp_t")
                    for s in range(4):
                        nc.tensor.transpose(
                            ptc[:, 128 * s:128 * (s + 1)],
                            p3c[s][:, 128 * t:128 * (t + 1)], cs["ident"][:])
                    sc = wp.tile([128, 512], F32R, tag="t2c", name=f"t2c_{img}_{t}")
                    nc.vector.tensor_copy(sc[:], ptc[:])
                    t2c.append(sc)

                # ---- P4 + color back + LEVEL + clamp + store ----
                for t in range(4):
                    for ci, cname in enumerate(("R", "G", "B")):
                        ps = pmm.tile([128, 512], F32, tag="mm", name="psmm_t")
                        nc.tensor.matmul(ps[:], cs["w4y"][:], t2y[t][:],
                                         start=True, stop=False)
                        nc.tensor.matmul(ps[:], cs[f"w4c{cname}"][:], t2c[t][:],
                                         start=False, stop=False)
                        nc.tensor.matmul(ps[:], cs["lones"][:], cs["ones"][:],
                                         start=False, stop=True)
                        og = sp.tile([128, 512], F32, tag="og", name=f"og_{img}_{t}_{ci}")
                        nc.vector.tensor_scalar(og[:], ps[:], 0.0, 1.0,
                                                OP.max, OP.min)
                        nc.sync.dma_start(
                            out_d[img, ci, 128 * t:128 * (t + 1), :], og[:])
    nc.compile()
    return nc


_NC_CACHE = None
TRACE = False
TRACE_DIR = None
LAST = None


def kernel(input, quantize):
    global _NC_CACHE, _CONST_SHAPES, LAST
    input = np.asarray(input, dtype=np.float32)
    quantize = np.asarray(quantize, dtype=np.float32)
    consts = _build_consts(quantize)
    if _CONST_SHAPES is None:
        _CONST_SHAPES = {k: v.shape for k, v in consts.items()}
    if _NC_CACHE is None:
        _NC_CACHE = _build_nc()
    nc = _NC_CACHE

    in_maps = []
    for core in range(N_CORES):
        shard = np.ascontiguousarray(
            input[core * IMG_PER_CORE:(core + 1) * IMG_PER_CORE])
        m = {"x": shard}
        m.update(consts)
        in_maps.append(m)
    kw = {}
    if TRACE:
        kw = dict(trace=True, tmpdir=TRACE_DIR)
    res = bass_utils.run_bass_kernel_spmd(nc, in_maps,
                                          core_ids=list(range(N_CORES)), **kw)
    LAST = res
    out = np.concatenate([res.results[i]["out"] for i in range(N_CORES)],
                         axis=0)
    return out.astype(np.float32)



# revision 9
# speedup vs baseline: 2.9758x; 2.9758x over previous
"""JPEG layer (nn_JpegLayer) Trainium2 Bass kernel, 8-core data parallel.

Pipeline per image (per core: 4 images of [3,512,512]):
  P1 : 3-accum matmuls fold RGB->YCC color mix + H-DCT (+ vertical 2x-pool
       for chroma); Y level shift (-sqrt8*L at DC rows) folds into the PSUM
       eviction bias on the scalar engine.  Chroma eviction pools W by a
       strided pair-add on DVE (output [128,256]).
  T1 : PE transposes -> [w, h'freq]
  P2 : W-DCT (0.5x folded for chroma W-pool) -> full 2D coeffs [w'', h']
  Q  : ey = d*(1/q) (DVE); round via +/-1.5*2^23 (DVE); dec = r*q -> bf16
  ITP: fused W-IDCT + transpose as plain matmuls with the dec block as
       stationary: psum[.,s] = dec_block^T @ blockdiag(D) (bf16).  Y gets
       +sqrt8*L DC bias on eviction (restores +LEVEL).
  P4 : H-IDCT (+v-upsample+color for chroma via PU) with the chroma rhs
       W-upsampled through a broadcast AP -> psum RGB
  out: DVE tensor_scalar clamp(0,1) psum->sbuf, DMA out.
"""
import sys
sys.path.insert(0, '/opt/trn_rl_repo')
import numpy as np
import concourse.bacc as bacc
import concourse.bass as bass
import concourse.mybir as mybir
import concourse.tile as tile
from concourse import bass_utils

N_CORES = 8
IMG_PER_CORE = 4
H = W = 512
LEVEL = np.float32(128.0 / 255.0)
SQRT8L = float(np.sqrt(8.0) * LEVEL)
C_ROUND = 12582912.0   # 1.5*2^23: (x+C)-C == round-half-even(x)
F32 = mybir.dt.float32
F32R = mybir.dt.float32r
BF16 = mybir.dt.bfloat16

RGB2YCC = np.array([[0.299, 0.587, 0.114],
                    [-0.168735892, -0.331264108, 0.5],
                    [0.5, -0.418687589, -0.081312411]], dtype=np.float32)
# YCC2RGB columns: Y col=[1,1,1]; cb col=[0,-0.344136286,1.772]; cr col=[1.402,-0.714136286,0]
CB_C = np.array([0.0, -0.344136286, 1.772], dtype=np.float32)
CR_C = np.array([1.402, -0.714136286, 0.0], dtype=np.float32)


def _dct8():
    i = np.arange(8)[:, None].astype(np.float64)
    j = np.arange(8)[None, :].astype(np.float64)
    m = np.sqrt(2.0 / 8) * np.cos(np.pi * (2 * j + 1) * i / 16.0)
    m[0, :] = 1.0 / np.sqrt(8.0)
    return m.astype(np.float32)


def _blockdiag(b, reps):
    r, c = b.shape
    out = np.zeros((r * reps, c * reps), dtype=np.float32)
    for k in range(reps):
        out[k * r:(k + 1) * r, k * c:(k + 1) * c] = b
    return out


def _build_consts(quantize):
    D = _dct8()
    BD_T = _blockdiag(D.T, 16)             # [128,128] lhsT: out=DCT along part.
    BD = _blockdiag(D, 16)                 # [128,128] lhsT: out=IDCT along part.
    # pooled fwd (H): PF[16b+2ii+dh, 8b+u] = D[u,ii]/2    [128, 64]
    pf8 = np.zeros((16, 8), dtype=np.float32)
    for ii in range(8):
        for dh in range(2):
            pf8[2 * ii + dh, :] = D[:, ii] * 0.5
    PF = _blockdiag(pf8, 8)                # [128, 64]
    # upsample inverse (H): PU[8b+u, 16b+2ii+dh] = D[u, ii]   [64, 128]
    pu8 = np.zeros((8, 16), dtype=np.float32)
    for jj in range(8):
        for dw in range(2):
            pu8[:, 2 * jj + dw] = D[:, jj]
    PU = _blockdiag(pu8, 8)                # [64, 128]

    consts = {}
    for c in range(3):
        consts[f"w1y{c}"] = RGB2YCC[0, c] * BD_T
        consts[f"w1c{c}"] = np.concatenate(
            [RGB2YCC[1, c] * PF, RGB2YCC[2, c] * PF], axis=1)  # [128,128]
    consts["ident"] = np.eye(128, dtype=np.float32)
    consts["w2y"] = BD_T
    consts["w2c"] = 0.5 * BD_T             # W-pool avg fold
    consts["w4y"] = BD
    for name, cb, cr in (("R", CB_C[0], CR_C[0]), ("G", CB_C[1], CR_C[1]),
                         ("B", CB_C[2], CR_C[2])):
        m = np.zeros((128, 128), dtype=np.float32)
        m[0:64, :] = cb * PU
        m[64:128, :] = cr * PU
        consts[f"w4c{name}"] = m

    # quant tables: q = round(quantize[0]*255)/255 (f32, all channels)
    q = (np.round(quantize[0].astype(np.float32) * np.float32(255.0))
         / np.float32(255.0)).astype(np.float32)
    rq = (1.0 / q.astype(np.float64)).astype(np.float32)
    consts["rqt"] = np.tile(rq.T, (16, 64)).astype(np.float32)   # [128,512]
    consts["qt"] = np.tile(q.T, (16, 64)).astype(np.float32)
    # per-partition DC bias vectors (level shift fold, Y plane only)
    bd1 = np.zeros((128, 1), dtype=np.float32)
    bd1[0::8, 0] = -SQRT8L
    consts["bias_d1"] = bd1
    bt2 = np.zeros((128, 1), dtype=np.float32)
    bt2[0::8, 0] = SQRT8L
    consts["bias_t2"] = bt2
    return consts


_CONST_SHAPES = None
# dtype per const tile in SBUF (DMA copies raw f32 bits unless noted)
_CONST_DT = {"rqt": F32, "qt": F32, "bias_d1": F32, "bias_t2": F32}


def _build_nc():
    nc = bacc.Bacc("TRN2", target_bir_lowering=False, debug=False,
                   enable_asserts=False, num_devices=N_CORES)
    x_d = nc.dram_tensor("x", [IMG_PER_CORE, 3, H, W], F32R,
                         kind="ExternalInput").ap()
    out_d = nc.dram_tensor("out", [IMG_PER_CORE, 3, H, W], F32,
                           kind="ExternalOutput").ap()
    bdw_bf_d = nc.dram_tensor("bdw_bf", [128, 128], BF16,
                              kind="ExternalInput").ap()
    cd = {}
    for name, shape in _CONST_SHAPES.items():
        cd[name] = nc.dram_tensor(name, list(shape),
                                  _CONST_DT.get(name, F32R),
                                  kind="ExternalInput").ap()

    ACT = mybir.ActivationFunctionType
    OP = mybir.AluOpType

    with tile.TileContext(nc) as tc:
        with tc.tile_pool(name="consts", bufs=1) as cp, \
             tc.tile_pool(name="xin", bufs=6) as xp, \
             tc.tile_pool(name="work", bufs=8) as wp, \
             tc.tile_pool(name="og", bufs=6) as ogp, \
             tc.tile_pool(name="psA", bufs=2, space="PSUM") as pA, \
             tc.tile_pool(name="psB", bufs=2, space="PSUM") as pB, \
             tc.tile_pool(name="psC", bufs=2, space="PSUM") as pC, \
             tc.tile_pool(name="psE", bufs=2, space="PSUM") as pE:

            cs = {}
            for name, shape in _CONST_SHAPES.items():
                cs[name] = cp.tile(list(shape), _CONST_DT.get(name, F32R),
                                   tag=f"c_{name}", name=f"c_{name}")
                nc.sync.dma_start(cs[name][:], cd[name])
            bdw_bf = cp.tile([128, 128], BF16, tag="c_bdwb", name="c_bdwb")
            nc.sync.dma_start(bdw_bf[:], bdw_bf_d)

            for img in range(IMG_PER_CORE):
                # ---- load RGB: one DMA per channel, [128, 4*512] ----
                X = []
                for c in range(3):
                    xt = xp.tile([128, 2048], F32R, tag="x", name=f"x_{img}_{c}")
                    src = x_d[img, c].rearrange("(t p) w -> p t w", t=4)
                    nc.sync.dma_start(xt[:], src)
                    X.append(xt)

                # ---- P1: color + H-DCT (+v-pool chroma) ----
                d1y, d1c = [], []
                for t in range(4):
                    psY = pA.tile([128, 512], F32, tag="p1", name="psY_t")
                    for c in range(3):
                        nc.tensor.matmul(psY[:], cs[f"w1y{c}"][:],
                                         X[c][:, 512 * t:512 * (t + 1)],
                                         start=(c == 0), stop=(c == 2))
                    ty = wp.tile([128, 512], F32R, tag="d1y", name=f"d1y_{img}_{t}")
                    nc.scalar.activation(ty[:], psY[:], ACT.Identity,
                                         bias=cs["bias_d1"][:, 0:1])
                    d1y.append(ty)

                    psC = pA.tile([128, 256], F32, tag="p1", name="psC_t")
                    for c in range(3):
                        xs = X[c][:, 512 * t:512 * (t + 1)]
                        nc.tensor.matmul(psC[:], cs[f"w1c{c}"][:], xs[:, 0::2],
                                         start=(c == 0), stop=False)
                        nc.tensor.matmul(psC[:], cs[f"w1c{c}"][:], xs[:, 1::2],
                                         start=False, stop=(c == 2))
                    tcc = wp.tile([128, 256], F32R, tag="d1c", name=f"d1c_{img}_{t}")
                    nc.scalar.activation(tcc[:], psC[:], ACT.Copy)
                    d1c.append(tcc)

                # ---- T1: PE transposes ----
                t1y, t1c = [], []
                for s in range(4):
                    pty = pB.tile([128, 512], F32R, tag="tp", name="pstp_t")
                    for t in range(4):
                        nc.tensor.transpose(
                            pty[:, 128 * t:128 * (t + 1)],
                            d1y[t][:, 128 * s:128 * (s + 1)], cs["ident"][:])
                    sy = wp.tile([128, 512], F32R, tag="t1y", name=f"t1y_{img}_{s}")
                    nc.scalar.activation(sy[:], pty[:], ACT.Copy)
                    t1y.append(sy)
                for s in range(2):
                    ptc = pB.tile([128, 512], F32R, tag="tp", name="pstpc_t")
                    for t in range(4):
                        nc.tensor.transpose(
                            ptc[:, 128 * t:128 * (t + 1)],
                            d1c[t][:, 128 * s:128 * (s + 1)], cs["ident"][:])
                    sc = wp.tile([128, 512], F32R, tag="t1c", name=f"t1c_{img}_{s}")
                    nc.scalar.activation(sc[:], ptc[:], ACT.Copy)
                    t1c.append(sc)

                # ---- P2 + quantize (Y: 4 tiles; C: 2 tiles) ----
                decy, decc = [], []
                for s in range(6):
                    ps = pC.tile([128, 512], F32, tag="mm2", name="ps2_t")
                    if s < 4:
                        nc.tensor.matmul(ps[:], cs["w2y"][:], t1y[s][:],
                                         start=True, stop=True)
                    else:
                        nc.tensor.matmul(ps[:], cs["w2c"][:], t1c[s - 4][:],
                                         start=True, stop=True)
                    ey = wp.tile([128, 512], F32, tag="ey", name=f"ey_{img}_{s}",
                                 bufs=4)
                    nc.vector.tensor_tensor(ey[:], ps[:], cs["rqt"][:], OP.mult)
                    nc.vector.tensor_scalar(ey[:], ey[:], C_ROUND, C_ROUND,
                                            OP.add, OP.subtract)
                    dt_ = wp.tile([128, 512], BF16, tag="dec", name=f"dec_{img}_{s}")
                    nc.vector.tensor_tensor(dt_[:], ey[:], cs["qt"][:], OP.mult)
                    (decy if s < 4 else decc).append(dt_)

                # ---- ITP: fused W-IDCT + transpose (dec block as lhsT) ----
                t2y, t2c = [], []
                for t in range(4):
                    pt = pB.tile([128, 512], F32, tag="tp", name="psit_t")
                    for s in range(4):
                        nc.tensor.matmul(pt[:, 128 * s:128 * (s + 1)],
                                         decy[s][:, 128 * t:128 * (t + 1)],
                                         bdw_bf[:], start=True, stop=True)
                    sy = wp.tile([128, 512], F32R, tag="t2y", name=f"t2y_{img}_{t}")
                    nc.scalar.activation(sy[:], pt[:], ACT.Identity,
                                         bias=cs["bias_t2"][:, 0:1])
                    t2y.append(sy)
                for t in range(4):
                    pt = pB.tile([128, 512], F32, tag="tp", name="psitc_t")
                    for s in range(2):
                        nc.tensor.matmul(pt[:, 128 * s:128 * (s + 1)],
                                         decc[s][:, 128 * t:128 * (t + 1)],
                                         bdw_bf[:], start=True, stop=True)
                    sc = wp.tile([128, 256], F32R, tag="t2c", name=f"t2c_{img}_{t}")
                    nc.scalar.activation(sc[:], pt[:, 0:256], ACT.Copy)
                    t2c.append(sc)

                # ---- P4: H-IDCT + color + clamp + store ----
                for t in range(4):
                    t2c_up = t2c[t][:].unsqueeze(2).broadcast_to([128, 256, 2])
                    for ci, cname in enumerate(("R", "G", "B")):
                        ps = pE.tile([128, 512], F32, tag="mm4", name="ps4_t")
                        nc.tensor.matmul(ps[:], cs["w4y"][:], t2y[t][:],
                                         start=True, stop=False)
                        nc.tensor.matmul(ps[:], cs[f"w4c{cname}"][:], t2c_up,
                                         start=False, stop=True)
                        og = ogp.tile([128, 512], F32, tag="og",
                                      name=f"og_{img}_{t}_{ci}")
                        nc.vector.tensor_scalar(og[:], ps[:], 0.0, 1.0,
                                                OP.max, OP.min)
                        nc.sync.dma_start(
                            out_d[img, ci, 128 * t:128 * (t + 1), :], og[:])
    nc.compile()
    return nc


_NC_CACHE = None
TRACE = False
TRACE_DIR = None
LAST = None


def kernel(input, quantize):
    global _NC_CACHE, _CONST_SHAPES, LAST
    input = np.asarray(input, dtype=np.float32)
    quantize = np.asarray(quantize, dtype=np.float32)
    consts = _build_consts(quantize)
    if _CONST_SHAPES is None:
        _CONST_SHAPES = {k: v.shape for k, v in consts.items()}
    if _NC_CACHE is None:
        _NC_CACHE = _build_nc()
    nc = _NC_CACHE

    import ml_dtypes
    bdw_bf = _blockdiag(_dct8(), 16).astype(ml_dtypes.bfloat16)

    in_maps = []
    for core in range(N_CORES):
        shard = np.ascontiguousarray(
            input[core * IMG_PER_CORE:(core + 1) * IMG_PER_CORE])
        m = {"x": shard, "bdw_bf": bdw_bf}
        m.update(consts)
        in_maps.append(m)
    kw = {}
    if TRACE:
        kw = dict(trace=True, tmpdir=TRACE_DIR)
    res = bass_utils.run_bass_kernel_spmd(nc, in_maps,
                                          core_ids=list(range(N_CORES)), **kw)
    LAST = res
    out = np.concatenate([res.results[i]["out"] for i in range(N_CORES)],
                         axis=0)
    return out.astype(np.float32)
